# revision 5
# baseline (speedup 1.0000x reference)
"""MoE routing kernel for Trainium2 (8 NeuronCores, data-parallel over tokens).

Reference computation:
    scores = x @ gate_w.T                   [N, E] (must be fp32-exact:
        bf16 scores would flip top-k selections -> full-expert-sized errors)
    top-4 experts per token; routing weight = top-1 score for ALL selected
    hs = sum_{e in top4} (x @ expert_w[e].T) * top1
    out = relu(hs)^2 @ out_w.T

Sharding: tokens split 1024/core, no collectives. Weights replicated;
host pre-formats them (transpose to contraction-major, fp8 hi/lo split, and
expert_w tiled [ii, d_in, e, dd, i_in] so each i-tile's load is one
contiguous DMA). Gate weights stay fp32.

Main-GEMM precision: 3-term fp8 residual emulation of bf16. Host splits
x*32 = xh + xl and W*1024 = Wh + Wl (each e4m3); device computes
    hs ~= xh@Wh + xh@Wl + xl@Wh          (xl@Wl ~ 2^-8 relative, dropped)
with fp8 DoubleRow matmuls (2 contraction slices per pass at 0.5 cyc/col),
i.e. 12 slice-pairs per 16 contraction slices = 0.75x the bf16 PE cost.
The 2^15 scale is folded into out_w on the host (relu^2 is degree-2
homogeneous, so scales pass through to the out-projection linearly).

Device pipeline per core (all engines overlapped by the Tile scheduler):
  1. PE fp32 transposes of x -> xT; fp32 gate GEMM -> scores; DVE evicts
     xh8 = e4m3(xT) and xl8 = e4m3(xT - xh8).
  2. DVE max8 sorts each token's 8 scores; threshold = col 3, top1 = col 0.
     The top-1 weight is split exactly: sign(top1) goes into the expert
     masks, top1^2 (fp32) is applied at the out-projection eviction.
  3. Mask rows: PE-transpose of the [t, e] mask + a rank-1 matmul per
     expert broadcasts each mask row to all 128 partitions; one DVE multiply
     per expert builds the masked operands z{h,l}_e = x{h,l}8 * maskrow_e.
  4. Main GEMM: per (chunk, i-tile) PSUM bank: Pool memsets the bank, then
     all fp8 DoubleRow MMs accumulate with start=False (two independent
     256-col column blocks per bank; a start=True would re-arm the whole
     2KB pending-zero region and corrupt the sibling block).
  5. relu^2 on ScalarE (fp32 relu from PSUM, square casts to bf16).
  6. Out-projection GEMM (bf16) + fp32 top1^2 scale at eviction.
"""

import numpy as np
import ml_dtypes

_CACHE = {}

P = 128
T, D, E, I, DO = 1024, 1024, 8, 2048, 1024
TT, DD, II = T // P, D // P, I // P          # 8, 8, 16
NCH = 2                                      # token chunks per core
TPC = TT // NCH                              # t-tiles per chunk (4)
TC = TPC * P                                 # tokens per chunk (512)
NCORES = 8
SX = 32.0                                    # x scale (folded on host)
SW = 1024.0                                  # expert_w scale (folded on host)


def _split_sync_waits(nc):
    """walrus in this container caps sync waits per instruction (and rejects
    any wait on Drain). Move excess waits onto injected same-engine NOPs
    placed immediately before the instruction - the engine blocks on the
    nops' waits first, so the ordering semantics are identical."""
    from concourse import mybir

    uid = 0
    for bb in nc.m.functions[0].blocks:
        insts = bb.instructions
        new = []
        changed = False
        for inst in insts:
            si = getattr(inst, "sync_info", None)
            waits = list(si.on_wait) if si is not None and si.on_wait else []
            keep = 0 if isinstance(inst, mybir.InstDrain) else 1
            if len(waits) > keep:
                moved, kept = waits[: len(waits) - keep], waits[len(waits) - keep:]
                si.on_wait = kept
                for w in moved:
                    nop = mybir.InstNoOp(
                        name=f"wsplit-{uid}",
                        engine=inst.engine,
                        bass_nofuse=True,
                        sync_info=mybir.SyncInfo(on_wait=[w], on_update=[]),
                    )
                    uid += 1
                    new.append(nop)
                changed = True
            new.append(inst)
        if changed:
            bb.instructions = new


def _build_nc(reps=1, split_waits=True):
    import contextlib

    import concourse.bass as bass
    import concourse.mybir as mybir
    import concourse.tile as tile
    from concourse.masks import make_identity

    f32 = mybir.dt.float32
    bf16 = mybir.dt.bfloat16
    fp8 = mybir.dt.float8e4

    nc = bass.Bass("TRN2", target_bir_lowering=False, debug=False)
    x_d = nc.dram_tensor("x", [T, D], f32, kind="ExternalInput")
    gwt_d = nc.dram_tensor("gwt", [D, E], f32, kind="ExternalInput")
    # expert weights pre-tiled on host: [ii, d_inner, e, dd, i_inner]; hi/lo
    # e4m3 halves of W*1024
    ewh_d = nc.dram_tensor("ewh", [II, P, E, DD, P], fp8, kind="ExternalInput")
    ewl_d = nc.dram_tensor("ewl", [II, P, E, DD, P], fp8, kind="ExternalInput")
    owt_d = nc.dram_tensor("owt", [I, DO], bf16, kind="ExternalInput")
    out_d = nc.dram_tensor("out", [T, DO], f32, kind="ExternalOutput")

    xr = x_d.rearrange("(tt p) d -> p tt d", p=P)
    outr = out_d.rearrange("(tt p) d -> p tt d", p=P)
    gwr = gwt_d.rearrange("(dd p) e -> p dd e", p=P)
    owr = owt_d.rearrange("(ii p) d -> p ii d", p=P)

    with tile.TileContext(nc) as tc:
        with (
            tc.tile_pool(name="const", bufs=1) as constp,
            tc.tile_pool(name="xp", bufs=3) as xp,
            tc.tile_pool(name="xtp", bufs=1) as xtp,
            tc.tile_pool(name="gate", bufs=2) as gatep,
            tc.tile_pool(name="x8p", bufs=1) as x8p,
            tc.tile_pool(name="gp", bufs=1) as gp,
            tc.tile_pool(name="zp", bufs=1) as zp,
            tc.tile_pool(name="ewp", bufs=2) as ewp,
            tc.tile_pool(name="hstp", bufs=1) as hstp,
            tc.tile_pool(name="rp", bufs=2) as rp,
            tc.tile_pool(name="obp", bufs=2) as obp,
            tc.tile_pool(name="ps_sm", bufs=2, space="PSUM") as pss,
            tc.tile_pool(name="ps_gate", bufs=1, space="PSUM") as psg,
            tc.tile_pool(name="ps_hs", bufs=3, space="PSUM") as psh,
            tc.tile_pool(name="ps_out", bufs=2, space="PSUM") as pso,
        ):
            ident32 = constp.tile([P, P], f32)
            make_identity(nc, ident32)
            # one-hot rows: onehot8[k, e, :] = (k == e); stationary operand of
            # the rank-1 matmul that broadcasts a mask row to all 128
            # partitions
            onehot8 = constp.tile([8, E, P], bf16)
            nc.gpsimd.memset(onehot8[:], 0.0)
            nc.gpsimd.affine_select(
                out=onehot8[:], in_=onehot8[:],
                compare_op=mybir.AluOpType.not_equal, fill=1.0, base=0,
                pattern=[[-1, E], [0, P]], channel_multiplier=1,
            )
            gw_sb = constp.tile([P, DD, E], f32)
            nc.sync.dma_start(gw_sb[:], gwr[:, :, :])
            # out_w load is emitted late (after phase 1) so the x loads it
            # gates the PE on are not queued behind this 4MB transfer
            ow_sb = constp.tile([P, II, DO], bf16)

            wm_all = constp.tile([P, TT, E], f32)
            t1sq = constp.tile([P, TT], f32)
            xhl8 = x8p.tile([P, 2, DD, T], fp8)

            # reps>1 wraps the body in a device-side loop: used only for
            # timing (the body is idempotent), never for grading runs.
            loop_cm = (
                tc.For_i(
                    0, reps, 1,
                    hint_engines=(
                        mybir.EngineType.PE, mybir.EngineType.DVE,
                        mybir.EngineType.Activation, mybir.EngineType.SP,
                        mybir.EngineType.Pool,
                    ),
                )
                if reps > 1 else contextlib.nullcontext()
            )
            with loop_cm:
                _emit_body(
                    nc, tc, mybir, xr, outr, ewh_d, ewl_d, owr, gw_sb, ow_sb,
                    ident32, onehot8, wm_all, t1sq, xhl8, xp, xtp, gatep, gp,
                    zp, ewp, hstp, rp, obp, pss, psg, psh, pso,
                )
    if split_waits:
        _split_sync_waits(nc)
    return nc


def _emit_body(
    nc, tc, mybir, xr, outr, ewh_d, ewl_d, owr, gw_sb, ow_sb, ident32,
    onehot8, wm_all, t1sq, xhl8, xp, xtp, gatep, gp, zp, ewp, hstp, rp, obp,
    pss, psg, psh, pso,
):
    f32 = mybir.dt.float32
    bf16 = mybir.dt.bfloat16
    fp8 = mybir.dt.float8e4
    Alu = mybir.AluOpType
    Act = mybir.ActivationFunctionType
    DR = mybir.MatmulPerfMode.DoubleRow

    def emit_phase1(tt):
        xt = xp.tile([P, D], f32, tag="xtile")
        nc.sync.dma_start(xt[:], xr[:, tt, :])
        xTt = xtp.tile([P, DD, P], f32, tag="xT")
        for db in range(DD // 4):
            tp = pss.tile([P, 4 * P], f32, tag="sm")
            for dq in range(4):
                dd = db * 4 + dq
                nc.tensor.transpose(
                    tp[:, dq * P:(dq + 1) * P],
                    xt[:, dd * P:(dd + 1) * P], ident32[:],
                )
            nc.vector.tensor_copy(
                xTt[:, db * 4:(db + 1) * 4, :], tp[:]
            )
            for dq in range(4):
                dd = db * 4 + dq
                # hi/lo e4m3 split of xT (x pre-scaled by 32 on host)
                nc.vector.tensor_copy(
                    xhl8[:, 0, dd, tt * P:(tt + 1) * P],
                    tp[:, dq * P:(dq + 1) * P],
                )
                nc.vector.tensor_tensor(
                    xhl8[:, 1, dd, tt * P:(tt + 1) * P],
                    tp[:, dq * P:(dq + 1) * P],
                    xhl8[:, 0, dd, tt * P:(tt + 1) * P],
                    Alu.subtract,
                )
        gps = psg.tile([P, E], f32, tag="gate")
        for dd in range(DD):
            nc.tensor.matmul(
                gps[:], xTt[:, dd, :], gw_sb[:, dd, :],
                start=(dd == 0), stop=(dd == DD - 1),
            )
        sc = gatep.tile([P, E], f32, tag="sc")
        nc.vector.tensor_copy(sc[:], gps[:])
        s8 = gatep.tile([P, 8], f32, tag="s8")
        nc.vector.max(s8[:], sc[:])
        # split the top-1 weight into sign (exact in fp8, goes into the mask
        # rows) and top1^2 (fp32, applied at the out-proj eviction):
        # relu(w*h)^2 == w^2 * relu(sign(w)*h)^2 exactly.
        sgn = gatep.tile([P, 1], f32, tag="sgn")
        nc.vector.tensor_scalar(
            sgn[:], s8[:, 0:1], 0.0, None, Alu.is_ge
        )
        nc.vector.tensor_scalar(
            sgn[:], sgn[:], 2.0, -1.0, Alu.mult, Alu.add
        )
        nc.vector.tensor_scalar(
            wm_all[:, tt, :], sc[:], s8[:, 3:4], sgn[:, 0:1],
            Alu.is_ge, Alu.mult,
        )
        nc.vector.tensor_scalar(
            t1sq[:, tt:tt + 1], s8[:, 0:1], s8[:, 0:1], None, Alu.mult
        )

    # ---- Phase 1 for chunk 0 up front; chunk 1's gating is emitted after
    # chunk 0's main GEMM so its x loads/transposes overlap it.
    for tt in range(TPC):
        emit_phase1(tt)
    # out_w load deferred here so phase-1 x loads are not queued behind it
    nc.sync.dma_start(ow_sb[:], owr[:, :, :])

    # ---- Phase 2: per token-chunk: masked operands, expert GEMM, relu^2,
    #      out projection
    for ch in range(NCH):
        if ch + 1 < NCH:
            for tt in range((ch + 1) * TPC, (ch + 2) * TPC):
                emit_phase1(tt)
        # expert-mask rows first: transpose wm [t,e] -> [e,t], then a rank-1
        # matmul per expert broadcasts the row to all partitions.
        wps = psg.tile([P, TC], f32, tag="gate")
        for tl in range(TPC):
            tt = ch * TPC + tl
            nc.tensor.transpose(
                wps[:E, tl * P:(tl + 1) * P], wm_all[:, tt, :],
                ident32[:],
            )
        wmT16 = gatep.tile([8, TC], bf16, tag="wmT16")
        nc.vector.tensor_copy(wmT16[:E], wps[:E])
        wrow = gp.tile([P, E, TC], fp8, tag="wrow")
        for e in range(E):
            bps = pss.tile([P, TC], f32, tag="sm")
            nc.tensor.matmul(
                bps[:], onehot8[:, e, :], wmT16[:E],
                start=True, stop=True,
            )
            nc.vector.tensor_copy(wrow[:, e, :], bps[:])
        z8 = zp.tile([P, E, 2, DD, TC], fp8, tag="z8")
        for e in range(E):
            nc.vector.tensor_tensor(
                z8[:, e, :, :, :],
                xhl8[:, :, :, ch * TC:(ch + 1) * TC],
                wrow[:, e, None, None, :].to_broadcast([P, 2, DD, TC]),
                Alu.mult,
            )

        hst = hstp.tile([P, II, TC], bf16, tag="hst")
        for ii in range(II):
            ewh = ewp.tile([P, E, DD, P], fp8, tag="ewh")
            nc.sync.dma_start(ewh[:], ewh_d[ii])
            ewl = ewp.tile([P, E, DD, P], fp8, tag="ewl")
            nc.sync.dma_start(ewl[:], ewl_d[ii])
            hps = psh.tile([P, TC], f32, tag="hps")
            # all main-GEMM MMs run start=False into a Pool-zeroed bank: two
            # independent 256-col blocks share the bank and a start=True
            # would re-arm the whole 2KB pending-zero region.
            nc.vector.memset(hps[:], 0.0)
            # e-outer: the first MMs consume only expert 0's z slices, giving
            # each DVE z-multiply a runway at chunk starts
            for e in range(E):
                for cb in range(2):
                    cs = slice(cb * 256, (cb + 1) * 256)
                    for p2 in range(DD // 2):
                        ds = slice(2 * p2, 2 * p2 + 2)
                        for prod, (hi_lo, wt) in enumerate(
                            ((0, ewh), (0, ewl), (1, ewh))
                        ):
                            last = (
                                e == E - 1 and p2 == DD // 2 - 1 and prod == 2
                            )
                            nc.tensor.matmul(
                                hps[:, cs],
                                wt[:, e, ds, :],
                                z8[:, e, hi_lo, ds, cs],
                                start=False, stop=last,
                                perf_mode=DR, skip_group_check=True,
                            )
            rt = rp.tile([P, TC], f32, tag="rt")
            nc.scalar.activation(rt[:], hps[:], Act.Relu)
            nc.scalar.activation(hst[:, ii, :], rt[:], Act.Square)

        for tl in range(TPC):
            tt = ch * TPC + tl
            for dc in range(2):
                ops = pso.tile([P, 512], f32, tag="ops")
                for ii in range(II):
                    nc.tensor.matmul(
                        ops[:], hst[:, ii, tl * P:(tl + 1) * P],
                        ow_sb[:, ii, dc * 512:(dc + 1) * 512],
                        start=(ii == 0), stop=(ii == II - 1),
                    )
                ob = obp.tile([P, 512], f32, tag="ob")
                nc.vector.tensor_scalar(
                    ob[:], ops[:], t1sq[:, tt:tt + 1], None, Alu.mult
                )
                nc.sync.dma_start(
                    outr[:, tt, dc * 512:(dc + 1) * 512], ob[:]
                )


def _get_nc():
    if "nc" not in _CACHE:
        _CACHE["nc"] = _build_nc()
    return _CACHE["nc"]


def _make_in_maps(inputs):
    x = inputs["x"]
    top_k = int(inputs["top_k"])
    assert top_k == 4, f"kernel hardcodes top_k=4, got {top_k}"
    gate_w, expert_w, out_w = inputs["gate_w"], inputs["expert_w"], inputs["out_w"]
    B, S, Dm = x.shape
    assert (Dm, gate_w.shape[0], expert_w.shape[1], out_w.shape[0]) == (D, E, I, DO)
    xf = np.ascontiguousarray(
        np.asarray(x, dtype=np.float32).reshape(-1, Dm) * np.float32(SX)
    )
    assert xf.shape[0] == NCORES * T

    bf = ml_dtypes.bfloat16
    e4 = ml_dtypes.float8_e4m3
    gwt = np.ascontiguousarray(
        np.asarray(gate_w, np.float32).T / np.float32(SX)
    )                                                                     # [D, E]
    # [E, I, D] -> [II, d_inner, E, DD, i_inner] (pre-tiled for contiguous
    # DMA), W*1024 split into hi/lo e4m3 halves
    ws = (
        np.asarray(expert_w, np.float32)
        .reshape(E, II, P, DD, P)
        .transpose(1, 4, 0, 3, 2)
    ) * np.float32(SW)
    ws = np.clip(ws, -240.0, 240.0)
    ewh = np.ascontiguousarray(ws).astype(e4)
    ewl = np.clip(ws - ewh.astype(np.float32), -240.0, 240.0).astype(e4)
    ewl = np.ascontiguousarray(ewl)
    # fold 1/(SX*SW)^2 into out_w (exact: power-of-two exponent shift)
    owt = np.ascontiguousarray(
        np.asarray(out_w, np.float32).T / np.float32(SX * SW) ** 2
    ).astype(bf)                                                          # [I, DO]

    return [
        {"x": xf[c * T:(c + 1) * T], "gwt": gwt, "ewh": ewh, "ewl": ewl,
         "owt": owt}
        for c in range(NCORES)
    ]


def kernel(x, gate_w, expert_w, out_w, top_k):
    from concourse.bass_utils import run_bass_kernel_spmd

    in_maps = _make_in_maps(dict(
        x=x, gate_w=gate_w, expert_w=expert_w, out_w=out_w, top_k=top_k
    ))
    nc = _get_nc()
    res = run_bass_kernel_spmd(nc, in_maps, list(range(NCORES)))
    out = np.concatenate([res.results[c]["out"] for c in range(NCORES)], axis=0)
    B, S, Dm = x.shape
    return out.reshape(B, S, Dm).astype(np.float32)
